# revision 3
# baseline (speedup 1.0000x reference)
"""Joint Maximum Mean Discrepancy loss on 8 Trainium2 NeuronCores.

Math: for streams (s0,t0) and (s1,t1), the reference builds per-stream
Gaussian kernels K_r = exp(-gamma_r * dist_r) over feats_r = [src; tgt]
(N=8192 rows), takes their elementwise product, and returns
mean(s2s + t2t - 2*s2t) over the B x B blocks.

Device decomposition (v2 — fp8 DoubleRow + DVE reduce):
  exponent E_ij = 2*W_i.W_j - c_i - c_j with W = [sqrt(g0)*X0,
  sqrt(g1)*X1] (N x 320), c_i = |W_i|^2. Split c = cbar + delta and
  scale everything by 16 so the fp8 operands sit in e4m3's normal
  range:
    P_ij = (8W0_i).(4W0_j) + (8W1_i).(4W1_j) - 16*delta_i - 16*delta_j
    K_ij = exp(P_ij/16 - 2*cbar)
  The 256 stream-0 rows go through ONE fp8e4 DoubleRow matmul per
  m-tile (K=256 virtual in a single 512-col pass, ~2x bf16); the
  stream-1 rows + two delta rows ride a 66-row bf16 matmul. -2*cbar is
  the activation's per-partition bias AP and the 1/16 its scale, so the
  scalar engine applies them for free inside exp. Quantizing W in fp8
  perturbs source and target features identically, so the MMD
  difference cancels the quantization error (measured end-to-end rel
  err ~2e-6, better than all-bf16).

  Symmetry halves the work via a block-cyclic cover: core k owns
  row-chunks {k, k+8} (chunk = 512 rows) and computes 17 [512 x 512]
  blocks — column offsets d=0..8 from row chunk k, d=0..7 from chunk
  k+8 — every unordered off-diagonal chunk pair once (weight 2),
  diagonals once (weight 1). Per block: 8 matmuls (4 m-tiles x
  fp8-DR + bf16) into a 4-bank PSUM tile, one Exp activation over
  [128, 2048] (no accum_out — the idle vector engine does the
  [128,2048]->[128,1] block sum into acc instead, keeping the scalar
  engine's per-block cost at the pure-exp floor). Host applies
  weights/signs and the final reduction in float64.

Per-core device program (SPMD — identical instructions, data differs):
  - lhs8 [2, 128, 2, 512] fp8e4 : DoubleRow stationary, [g][p,t,r] =
    8*W0[gbase+r, t*128+p]
  - lhsb [2, 66, 512] bf16     : [8*W1^T ; ones ; -16*delta]
  - rhs8 [8, 128, 2, 2, 512] fp8e4 : chunk-PAIR tiles (one DMA each),
    [j][p,u,t,n] = 4*W0[(2j+u)*512+n, t*128+p], chunk order rotated by
    k so the program's chunk index is core-local
  - rhsb [8, 66, 2, 512] bf16  : [4*W1^T ; -16*delta ; ones]
  - bias [128, 1] f32 = -2*cbar
  - out "acc" [128, 17] f32
"""

import os

import numpy as np
import ml_dtypes

import concourse.bacc as bacc
import concourse.bass as bass
import concourse.mybir as mybir
import concourse.tile as tile
from concourse.bass_utils import run_bass_kernel_spmd

B = 4096
D0, D1 = 256, 64
N = 2 * B
CH = 512          # rows per chunk
NCHUNK = 16
NCORE = 8
MT = 128          # m-tile rows / partition count
NMT = CH // MT    # m-tiles per row-chunk (4)
NBLK = 17         # blocks per core (9 from chunk k, 8 from chunk k+8)
NCOL = NBLK       # acc columns
KB = D1 + 2       # bf16 contraction rows: 64 stream-1 + delta_j + delta_i
LAM = 4.0         # fp8 range scale on each operand; exp rescales by 1/16

F8 = ml_dtypes.float8_e4m3
BF = ml_dtypes.bfloat16

_N_WARMUP = int(os.environ.get("JMMD_WARMUP", "28"))

LAST_EXEC_NS = None
LAST_RESULTS = None

_CACHE: dict = {}


def _build():
    if "nc" in _CACHE:
        return _CACHE["nc"]
    nc = bacc.Bacc(
        "TRN2", target_bir_lowering=False, debug=False, enable_asserts=False
    )
    f32 = mybir.dt.float32
    bf16 = mybir.dt.bfloat16
    f8 = mybir.dt.float8e4
    DR = mybir.MatmulPerfMode.DoubleRow

    lhs8_d = nc.dram_tensor("lhs8", [2, MT, 2, CH], f8, kind="ExternalInput").ap()
    lhsb_d = nc.dram_tensor("lhsb", [2, KB, CH], bf16, kind="ExternalInput").ap()
    rhs8_d = nc.dram_tensor("rhs8", [8, MT, 2, 2, CH], f8, kind="ExternalInput").ap()
    rhsb_d = nc.dram_tensor("rhsb", [8, KB, 2, CH], bf16, kind="ExternalInput").ap()
    bias_d = nc.dram_tensor("bias", [MT, 1], f32, kind="ExternalInput").ap()
    acc_d = nc.dram_tensor("acc", [MT, NCOL], f32, kind="ExternalOutput").ap()

    with tile.TileContext(nc) as tc:
        with (
            tc.tile_pool(name="const", bufs=1) as const,
            tc.tile_pool(name="psum", bufs=2, space=bass.MemorySpace.PSUM) as psum,
        ):
            # warmup scratch memset goes FIRST on gpsimd — anything queued
            # behind bulk DMAs on that engine would stall the PE program.
            scratch = None
            if _N_WARMUP:
                scratch = const.tile([MT, 256], bf16, tag="warm_src")
                nc.gpsimd.memset(scratch[:], 0.0)
            bias_t = const.tile([MT, 1], f32, tag="bias")
            nc.gpsimd.dma_start(bias_t[:], bias_d[:, :])

            l8, lb, r8, rb = {}, {}, {}, {}

            def load_lhs(g, eng):
                t8 = const.tile([MT, 2, CH], f8, tag=f"l8_{g}")
                eng.dma_start(t8[:], lhs8_d[g])
                l8[g] = t8
                tb = const.tile([KB, CH], bf16, tag=f"lb_{g}")
                eng.dma_start(tb[:], lhsb_d[g])
                lb[g] = tb

            def load_rhs(j, eng):
                t8 = const.tile([MT, 2, 2, CH], f8, tag=f"r8_{j}")
                eng.dma_start(t8[:], rhs8_d[j])
                r8[j] = t8
                tb = const.tile([KB, 2, CH], bf16, tag=f"rb_{j}")
                eng.dma_start(tb[:], rhsb_d[j])
                rb[j] = tb

            # block 0's operands race down both DMA engines in parallel;
            # later chunk pairs alternate engines in consumption order.
            load_lhs(0, nc.sync)
            load_rhs(0, nc.gpsimd)
            load_rhs(1, nc.sync)
            load_rhs(2, nc.gpsimd)
            load_rhs(3, nc.sync)
            load_rhs(4, nc.gpsimd)
            load_lhs(1, nc.sync)
            load_rhs(5, nc.sync)
            load_rhs(6, nc.gpsimd)
            load_rhs(7, nc.sync)

            acc_t = const.tile([MT, NCOL], f32, tag="acc")

            # HAM warmup: dense dummy matmuls while input DMAs stream, so
            # real matmuls start at the warm PE clock.
            if _N_WARMUP:
                warm_ps = psum.tile([MT, NMT * CH], f32, tag="ps")
                for _ in range(_N_WARMUP):
                    nc.tensor.matmul(
                        warm_ps[:, :MT],
                        scratch[:, :MT],
                        scratch[:, MT:],
                        start=True,
                        stop=True,
                    )

            for g, nd in ((0, 9), (1, 8)):
                for d in range(nd):
                    ch = d if g == 0 else 8 + d
                    col = d if g == 0 else 9 + d
                    j, u = divmod(ch, 2)
                    ps = psum.tile([MT, NMT * CH], f32, tag="ps")
                    for m in range(NMT):
                        nc.tensor.matmul(
                            ps[:, m * CH:(m + 1) * CH],
                            l8[g][:, :, m * MT:(m + 1) * MT],
                            r8[j][:, u],
                            start=True,
                            stop=False,
                            perf_mode=DR,
                        )
                        nc.tensor.matmul(
                            ps[:, m * CH:(m + 1) * CH],
                            lb[g][:, m * MT:(m + 1) * MT],
                            rb[j][:, u],
                            start=False,
                            stop=True,
                        )
                    nc.scalar.activation(
                        ps[:],
                        ps[:],
                        mybir.ActivationFunctionType.Exp,
                        bias=bias_t[:, 0:1],
                        scale=1.0 / (LAM * LAM),
                    )
                    nc.vector.tensor_reduce(
                        acc_t[:, col:col + 1],
                        ps[:],
                        axis=mybir.AxisListType.X,
                        op=mybir.AluOpType.add,
                    )
            nc.sync.dma_start(acc_d[:], acc_t[:])
    nc.compile()
    _CACHE["nc"] = nc
    return nc


def _pack_inputs(s0, s1, t0, t1):
    X0 = np.concatenate([s0, t0], axis=0).astype(np.float64)
    X1 = np.concatenate([s1, t1], axis=0).astype(np.float64)

    def gamma_of(X):
        sq = np.sum(X * X, axis=1)
        sdist = 2.0 * X.shape[0] * np.sum(sq) - 2.0 * np.sum(np.sum(X, axis=0) ** 2)
        return (X.shape[0] ** 2 - X.shape[0]) / sdist, sq

    g0, sq0 = gamma_of(X0)
    g1, sq1 = gamma_of(X1)
    c = g0 * sq0 + g1 * sq1
    cbar = c.mean()
    delta16 = -16.0 * (c - cbar)
    W0 = np.sqrt(g0) * X0  # [N, 256]
    W1 = np.sqrt(g1) * X1  # [N, 64]

    L0 = np.asarray(2.0 * LAM * W0, dtype=F8)  # [N, 256] lhs fp8
    R0 = np.asarray(LAM * W0, dtype=F8)        # [N, 256] rhs fp8
    # global rhs fp8 staged [ch, p, t, n]: feature f = t*128+p
    G8 = np.ascontiguousarray(R0.reshape(NCHUNK, CH, 2, MT).transpose(0, 3, 2, 1))
    # global rhs bf16 [ch, k, n]
    Gb = np.empty((NCHUNK, KB, CH), dtype=np.float64)
    for ch in range(NCHUNK):
        rows = slice(ch * CH, (ch + 1) * CH)
        Gb[ch, :D1] = LAM * W1[rows].T
        Gb[ch, D1] = delta16[rows]
        Gb[ch, D1 + 1] = 1.0
    Gb = Gb.astype(BF)

    def lhs_for(chunk):
        rows = slice(chunk * CH, (chunk + 1) * CH)
        a8 = np.ascontiguousarray(L0[rows].reshape(CH, 2, MT).transpose(2, 1, 0))
        ab = np.empty((KB, CH), dtype=np.float64)
        ab[:D1] = 2.0 * LAM * W1[rows].T
        ab[D1] = 1.0
        ab[D1 + 1] = delta16[rows]
        return a8, ab.astype(BF)

    bias = np.full((MT, 1), -2.0 * cbar, dtype=np.float32)

    in_maps = []
    for k in range(NCORE):
        a80, ab0 = lhs_for(k)
        a81, ab1 = lhs_for((k + 8) % NCHUNK)
        order = [(k + d) % NCHUNK for d in range(NCHUNK)]
        r8 = G8[order].reshape(8, 2, MT, 2, CH).transpose(0, 2, 1, 3, 4)
        rb = Gb[order].reshape(8, 2, KB, CH).transpose(0, 2, 1, 3)
        in_maps.append({
            "lhs8": np.ascontiguousarray(np.stack([a80, a81])),
            "lhsb": np.ascontiguousarray(np.stack([ab0, ab1])),
            "rhs8": np.ascontiguousarray(r8),
            "rhsb": np.ascontiguousarray(rb),
            "bias": bias,
        })
    return in_maps


def _combine(results):
    sgn = lambda ch: 1.0 if ch < NCHUNK // 2 else -1.0
    total = 0.0
    for k in range(NCORE):
        acc = np.asarray(results[k]["acc"], dtype=np.float64)  # [128, 17]
        colsum = acc.sum(axis=0)
        for col in range(NCOL):
            if col < 9:
                d, row_chunk = col, k
            else:
                d, row_chunk = col - 9, (k + 8) % NCHUNK
            col_chunk = (row_chunk + d) % NCHUNK
            w = 1.0 if d == 0 else 2.0
            s = sgn(row_chunk) * sgn(col_chunk)
            total += w * s * colsum[col]
    return total / (B * B)


def kernel(s0, s1, t0, t1):
    global LAST_EXEC_NS, LAST_RESULTS
    nc = _build()
    in_maps = _pack_inputs(
        np.asarray(s0), np.asarray(s1), np.asarray(t0), np.asarray(t1)
    )
    trace = os.environ.get("JMMD_TRACE", "0") == "1"
    res = run_bass_kernel_spmd(nc, in_maps, core_ids=list(range(NCORE)), trace=trace)
    LAST_EXEC_NS = res.exec_time_ns
    LAST_RESULTS = res
    return np.float32(_combine(res.results))


# revision 6
# speedup vs baseline: 1.1293x; 1.1293x over previous
"""Joint Maximum Mean Discrepancy loss on 8 Trainium2 NeuronCores.

Math: for streams (s0,t0) and (s1,t1), the reference builds per-stream
Gaussian kernels K_r = exp(-gamma_r * dist_r) over feats_r = [src; tgt]
(N=8192 rows), takes their elementwise product, and returns
mean(s2s + t2t - 2*s2t) over the B x B blocks.

Device decomposition (v2 — fp8 DoubleRow + DVE reduce):
  exponent E_ij = 2*W_i.W_j - c_i - c_j with W = [sqrt(g0)*X0,
  sqrt(g1)*X1] (N x 320), c_i = |W_i|^2. Split c = cbar + delta and
  scale everything by 16 so the fp8 operands sit in e4m3's normal
  range:
    P_ij = (8W0_i).(4W0_j) + (8W1_i).(4W1_j) - 16*delta_i - 16*delta_j
    K_ij = exp(P_ij/16 - 2*cbar)
  The 256 stream-0 rows go through ONE fp8e4 DoubleRow matmul per
  m-tile (K=256 virtual in a single 512-col pass, ~2x bf16); the
  stream-1 rows + two delta rows ride a 66-row bf16 matmul. -2*cbar is
  the activation's per-partition bias AP and the 1/16 its scale, so the
  scalar engine applies them for free inside exp. Quantizing W in fp8
  perturbs source and target features identically, so the MMD
  difference cancels the quantization error (measured end-to-end rel
  err ~2e-6, better than all-bf16).

  Symmetry halves the work via a block-cyclic cover: core k owns
  row-chunks {k, k+8} (chunk = 512 rows) and computes 17 [512 x 512]
  blocks — column offsets d=0..8 from row chunk k, d=0..7 from chunk
  k+8 — every unordered off-diagonal chunk pair once (weight 2),
  diagonals once (weight 1). Per block: 8 matmuls (4 m-tiles x
  fp8-DR + bf16) into a 4-bank PSUM tile, one Exp activation over
  [128, 2048] (no accum_out — the idle vector engine does the
  [128,2048]->[128,1] block sum into acc instead, keeping the scalar
  engine's per-block cost at the pure-exp floor). Host applies
  weights/signs and the final reduction in float64.

Per-core device program (SPMD — identical instructions, data differs):
  - lhs8 [2, 128, 2, 512] fp8e4 : DoubleRow stationary, [g][p,t,r] =
    8*W0[gbase+r, t*128+p]
  - lhsb [2, 66, 512] bf16     : [8*W1^T ; ones ; -16*delta]
  - rhs8 [8, 128, 2, 2, 512] fp8e4 : chunk-PAIR tiles (one DMA each),
    [j][p,u,t,n] = 4*W0[(2j+u)*512+n, t*128+p], chunk order rotated by
    k so the program's chunk index is core-local
  - rhsb [8, 66, 2, 512] bf16  : [4*W1^T ; -16*delta ; ones]
  - bias [128, 1] f32 = -2*cbar
  - out "acc" [128, 17] f32
"""

import os

import numpy as np
import ml_dtypes

import concourse.bacc as bacc
import concourse.bass as bass
import concourse.mybir as mybir
import concourse.tile as tile
from concourse.bass_utils import run_bass_kernel_spmd

B = 4096
D0, D1 = 256, 64
N = 2 * B
CH = 512          # rows per chunk
NCHUNK = 16
NCORE = 8
MT = 128          # m-tile rows / partition count
NMT = CH // MT    # m-tiles per row-chunk (4)
NBLK = 17         # blocks per core (9 from chunk k, 8 from chunk k+8)
NCOL = NBLK       # acc columns
KB = D1 + 2       # bf16 contraction rows: 64 stream-1 + delta_j + delta_i
LAM = 4.0         # fp8 range scale on each operand; exp rescales by 1/16

F8 = ml_dtypes.float8_e4m3
BF = ml_dtypes.bfloat16

_N_WARMUP = int(os.environ.get("JMMD_WARMUP", "32"))

LAST_EXEC_NS = None
LAST_RESULTS = None

_CACHE: dict = {}


def _build():
    if "nc" in _CACHE:
        return _CACHE["nc"]
    nc = bacc.Bacc(
        "TRN2", target_bir_lowering=False, debug=False, enable_asserts=False
    )
    f32 = mybir.dt.float32
    bf16 = mybir.dt.bfloat16
    f8 = mybir.dt.float8e4
    DR = mybir.MatmulPerfMode.DoubleRow

    lhs8_d = nc.dram_tensor("lhs8", [2, MT, 2, CH], f8, kind="ExternalInput").ap()
    lhsb_d = nc.dram_tensor("lhsb", [2, KB, CH], bf16, kind="ExternalInput").ap()
    rhs8_d = nc.dram_tensor("rhs8", [8, MT, 2, 2, CH], f8, kind="ExternalInput").ap()
    rhsb_d = nc.dram_tensor("rhsb", [8, KB, 2, CH], bf16, kind="ExternalInput").ap()
    bias_d = nc.dram_tensor("bias", [MT, 1], f32, kind="ExternalInput").ap()
    acc_d = nc.dram_tensor("acc", [MT, NCOL], f32, kind="ExternalOutput").ap()

    with tile.TileContext(nc) as tc:
        with (
            tc.tile_pool(name="const", bufs=1) as const,
            tc.tile_pool(name="psum", bufs=2, space=bass.MemorySpace.PSUM) as psum,
        ):
            # warmup scratch memset goes FIRST on gpsimd — anything queued
            # behind bulk DMAs on that engine would stall the PE program.
            scratch = None
            if _N_WARMUP:
                scratch = const.tile([MT, 256], bf16, tag="warm_src")
                nc.gpsimd.memset(scratch[:], 0.0)
            bias_t = const.tile([MT, 1], f32, tag="bias")
            nc.gpsimd.dma_start(bias_t[:], bias_d[:, :])

            l8, lb, r8, rb = {}, {}, {}, {}

            def load_lhs(g, eng):
                t8 = const.tile([MT, 2, CH], f8, tag=f"l8_{g}")
                eng.dma_start(t8[:], lhs8_d[g])
                l8[g] = t8
                tb = const.tile([KB, CH], bf16, tag=f"lb_{g}")
                eng.dma_start(tb[:], lhsb_d[g])
                lb[g] = tb

            def load_rhs(j, eng):
                t8 = const.tile([MT, 2, 2, CH], f8, tag=f"r8_{j}")
                eng.dma_start(t8[:], rhs8_d[j])
                r8[j] = t8
                tb = const.tile([KB, 2, CH], bf16, tag=f"rb_{j}")
                eng.dma_start(tb[:], rhsb_d[j])
                rb[j] = tb

            # block 0's operands race down both DMA engines in parallel;
            # later chunk pairs alternate engines in consumption order.
            load_lhs(0, nc.sync)
            load_rhs(0, nc.gpsimd)
            load_rhs(1, nc.sync)
            load_rhs(2, nc.gpsimd)
            load_rhs(3, nc.sync)
            load_rhs(4, nc.gpsimd)
            load_lhs(1, nc.sync)
            load_rhs(5, nc.sync)
            load_rhs(6, nc.gpsimd)
            load_rhs(7, nc.sync)

            acc_t = const.tile([MT, NCOL], f32, tag="acc")
            # exp lands in SBUF (bf16) so the psum tile frees at ACT end —
            # keeping the PE dense (HAM stays warm) — and the vector
            # engine reduces at the 2x 16-bit rate off the critical path.
            exp_t = const.tile([MT, 2, NMT * CH], mybir.dt.float16, tag="exp")

            # HAM warmup: dense dummy matmuls while input DMAs stream, so
            # real matmuls start at the warm PE clock.
            if _N_WARMUP:
                warm_ps = psum.tile([MT, NMT * CH], f32, tag="ps")
                for _ in range(_N_WARMUP):
                    nc.tensor.matmul(
                        warm_ps[:, :MT],
                        scratch[:, :MT],
                        scratch[:, MT:],
                        start=True,
                        stop=True,
                    )

            for g, nd in ((0, 9), (1, 8)):
                for d in range(nd):
                    ch = d if g == 0 else 8 + d
                    col = d if g == 0 else 9 + d
                    j, u = divmod(ch, 2)
                    ps = psum.tile([MT, NMT * CH], f32, tag="ps")
                    for m in range(NMT):
                        nc.tensor.matmul(
                            ps[:, m * CH:(m + 1) * CH],
                            l8[g][:, :, m * MT:(m + 1) * MT],
                            r8[j][:, u],
                            start=True,
                            stop=False,
                            perf_mode=DR,
                        )
                        nc.tensor.matmul(
                            ps[:, m * CH:(m + 1) * CH],
                            lb[g][:, m * MT:(m + 1) * MT],
                            rb[j][:, u],
                            start=False,
                            stop=True,
                        )
                    slot = col % 2
                    nc.scalar.activation(
                        exp_t[:, slot],
                        ps[:],
                        mybir.ActivationFunctionType.Exp,
                        bias=bias_t[:, 0:1],
                        scale=1.0 / (LAM * LAM),
                    )
                    nc.vector.tensor_reduce(
                        acc_t[:, col:col + 1],
                        exp_t[:, slot],
                        axis=mybir.AxisListType.X,
                        op=mybir.AluOpType.add,
                    )
            nc.sync.dma_start(acc_d[:], acc_t[:])
    nc.compile()
    _CACHE["nc"] = nc
    return nc


def _pack_inputs(s0, s1, t0, t1):
    X0 = np.concatenate([s0, t0], axis=0).astype(np.float64)
    X1 = np.concatenate([s1, t1], axis=0).astype(np.float64)

    def gamma_of(X):
        sq = np.sum(X * X, axis=1)
        sdist = 2.0 * X.shape[0] * np.sum(sq) - 2.0 * np.sum(np.sum(X, axis=0) ** 2)
        return (X.shape[0] ** 2 - X.shape[0]) / sdist, sq

    g0, sq0 = gamma_of(X0)
    g1, sq1 = gamma_of(X1)
    c = g0 * sq0 + g1 * sq1
    cbar = c.mean()
    delta16 = -16.0 * (c - cbar)
    W0 = np.sqrt(g0) * X0  # [N, 256]
    W1 = np.sqrt(g1) * X1  # [N, 64]

    L0 = np.asarray(2.0 * LAM * W0, dtype=F8)  # [N, 256] lhs fp8
    R0 = np.asarray(LAM * W0, dtype=F8)        # [N, 256] rhs fp8
    # global rhs fp8 staged [ch, p, t, n]: feature f = t*128+p
    G8 = np.ascontiguousarray(R0.reshape(NCHUNK, CH, 2, MT).transpose(0, 3, 2, 1))
    # global rhs bf16 [ch, k, n]
    Gb = np.empty((NCHUNK, KB, CH), dtype=np.float64)
    for ch in range(NCHUNK):
        rows = slice(ch * CH, (ch + 1) * CH)
        Gb[ch, :D1] = LAM * W1[rows].T
        Gb[ch, D1] = delta16[rows]
        Gb[ch, D1 + 1] = 1.0
    Gb = Gb.astype(BF)

    def lhs_for(chunk):
        rows = slice(chunk * CH, (chunk + 1) * CH)
        a8 = np.ascontiguousarray(L0[rows].reshape(CH, 2, MT).transpose(2, 1, 0))
        ab = np.empty((KB, CH), dtype=np.float64)
        ab[:D1] = 2.0 * LAM * W1[rows].T
        ab[D1] = 1.0
        ab[D1 + 1] = delta16[rows]
        return a8, ab.astype(BF)

    bias = np.full((MT, 1), -2.0 * cbar, dtype=np.float32)

    in_maps = []
    for k in range(NCORE):
        a80, ab0 = lhs_for(k)
        a81, ab1 = lhs_for((k + 8) % NCHUNK)
        order = [(k + d) % NCHUNK for d in range(NCHUNK)]
        r8 = G8[order].reshape(8, 2, MT, 2, CH).transpose(0, 2, 1, 3, 4)
        rb = Gb[order].reshape(8, 2, KB, CH).transpose(0, 2, 1, 3)
        in_maps.append({
            "lhs8": np.ascontiguousarray(np.stack([a80, a81])),
            "lhsb": np.ascontiguousarray(np.stack([ab0, ab1])),
            "rhs8": np.ascontiguousarray(r8),
            "rhsb": np.ascontiguousarray(rb),
            "bias": bias,
        })
    return in_maps


def _combine(results):
    sgn = lambda ch: 1.0 if ch < NCHUNK // 2 else -1.0
    total = 0.0
    for k in range(NCORE):
        acc = np.asarray(results[k]["acc"], dtype=np.float64)  # [128, 17]
        colsum = acc.sum(axis=0)
        for col in range(NCOL):
            if col < 9:
                d, row_chunk = col, k
            else:
                d, row_chunk = col - 9, (k + 8) % NCHUNK
            col_chunk = (row_chunk + d) % NCHUNK
            w = 1.0 if d == 0 else 2.0
            s = sgn(row_chunk) * sgn(col_chunk)
            total += w * s * colsum[col]
    return total / (B * B)


def kernel(s0, s1, t0, t1):
    global LAST_EXEC_NS, LAST_RESULTS
    nc = _build()
    in_maps = _pack_inputs(
        np.asarray(s0), np.asarray(s1), np.asarray(t0), np.asarray(t1)
    )
    trace = os.environ.get("JMMD_TRACE", "0") == "1"
    res = run_bass_kernel_spmd(nc, in_maps, core_ids=list(range(NCORE)), trace=trace)
    LAST_EXEC_NS = res.exec_time_ns
    LAST_RESULTS = res
    return np.float32(_combine(res.results))


# revision 12
# speedup vs baseline: 1.6373x; 1.4499x over previous
"""Joint Maximum Mean Discrepancy loss on 8 Trainium2 NeuronCores.

Math: for streams (s0,t0) and (s1,t1), the reference builds per-stream
Gaussian kernels K_r = exp(-gamma_r * dist_r) over feats_r = [src; tgt]
(N=8192 rows), takes their elementwise product, and returns
mean(s2s + t2t - 2*s2t) over the B x B blocks.

Device decomposition (v2 — fp8 DoubleRow + DVE reduce):
  exponent E_ij = 2*W_i.W_j - c_i - c_j with W = [sqrt(g0)*X0,
  sqrt(g1)*X1] (N x 320), c_i = |W_i|^2. Split c = cbar + delta and
  scale everything by 16 so the fp8 operands sit in e4m3's normal
  range:
    P_ij = (8W0_i).(4W0_j) + (8W1_i).(4W1_j) - 16*delta_i - 16*delta_j
    K_ij = exp(P_ij/16 - 2*cbar)
  The 256 stream-0 rows go through ONE fp8e4 DoubleRow matmul per
  m-tile (K=256 virtual in a single 512-col pass, ~2x bf16); the
  stream-1 rows + two delta rows ride a 66-row bf16 matmul. -2*cbar is
  the activation's per-partition bias AP and the 1/16 its scale, so the
  scalar engine applies them for free inside exp. Quantizing W in fp8
  perturbs source and target features identically, so the MMD
  difference cancels the quantization error (measured end-to-end rel
  err ~2e-6, better than all-bf16).

  Symmetry halves the work via a block-cyclic cover: core k owns
  row-chunks {k, k+8} (chunk = 512 rows) and computes 17 [512 x 512]
  blocks — column offsets d=0..8 from row chunk k, d=0..7 from chunk
  k+8 — every unordered off-diagonal chunk pair once (weight 2),
  diagonals once (weight 1). Per block: 8 matmuls (4 m-tiles x
  fp8-DR + bf16) into a 4-bank PSUM tile, one Exp activation over
  [128, 2048] (no accum_out — the idle vector engine does the
  [128,2048]->[128,1] block sum into acc instead, keeping the scalar
  engine's per-block cost at the pure-exp floor). Host applies
  weights/signs and the final reduction in float64.

Per-core device program (SPMD — identical instructions, data differs):
  - lhs8 [2, 128, 2, 512] fp8e4 : DoubleRow stationary, [g][p,t,r] =
    8*W0[gbase+r, t*128+p]
  - lhsb [2, 66, 512] bf16     : [8*W1^T ; ones ; -16*delta]
  - rhs8 [8, 128, 2, 2, 512] fp8e4 : chunk-PAIR tiles (one DMA each),
    [j][p,u,t,n] = 4*W0[(2j+u)*512+n, t*128+p], chunk order rotated by
    k so the program's chunk index is core-local
  - rhsb [8, 66, 2, 512] bf16  : [4*W1^T ; -16*delta ; ones]
  - bias [128, 1] f32 = -2*cbar
  - out "acc" [128, 17] f32
"""

import os

import numpy as np
import ml_dtypes

import concourse.bacc as bacc
import concourse.bass as bass
import concourse.mybir as mybir
import concourse.tile as tile
from concourse.bass_utils import run_bass_kernel_spmd

B = 4096
D0, D1 = 256, 64
N = 2 * B
CH = 512          # rows per chunk
NCHUNK = 16
NCORE = 8
MT = 128          # m-tile rows / partition count
NMT = CH // MT    # m-tiles per row-chunk (4)
NBLK = 17         # blocks per core (9 from chunk k, 8 from chunk k+8)
NCOL = NBLK       # acc columns
KB = D1 + 2       # bf16 contraction rows: 64 stream-1 + delta_j + delta_i
LAM = 4.0         # fp8 range scale on each operand; exp rescales by 1/16

F8 = ml_dtypes.float8_e4m3
BF = ml_dtypes.bfloat16

_N_WARMUP = int(os.environ.get("JMMD_WARMUP", "36"))

LAST_EXEC_NS = None
LAST_RESULTS = None

_CACHE: dict = {}


def _build():
    if "nc" in _CACHE:
        return _CACHE["nc"]
    nc = bacc.Bacc(
        "TRN2", target_bir_lowering=False, debug=False, enable_asserts=False
    )
    f32 = mybir.dt.float32
    bf16 = mybir.dt.bfloat16
    f8 = mybir.dt.float8e4
    DR = mybir.MatmulPerfMode.DoubleRow

    lhs8_d = nc.dram_tensor("lhs8", [2, MT, 2, CH], f8, kind="ExternalInput").ap()
    lhsb_d = nc.dram_tensor("lhsb", [2, KB, CH], bf16, kind="ExternalInput").ap()
    rhs8_d = nc.dram_tensor("rhs8", [8, MT, 2, 2, CH], f8, kind="ExternalInput").ap()
    rhsb_d = nc.dram_tensor("rhsb", [8, KB, 2, CH], bf16, kind="ExternalInput").ap()
    bias_d = nc.dram_tensor("bias", [MT, 1], f32, kind="ExternalInput").ap()
    acc_d = nc.dram_tensor("acc", [MT, NCOL], f32, kind="ExternalOutput").ap()

    with tile.TileContext(nc) as tc:
        with (
            tc.tile_pool(name="const", bufs=1) as const,
            tc.tile_pool(name="psum", bufs=2, space=bass.MemorySpace.PSUM) as psum,
        ):
            # warmup scratch memset on the idle vector engine so both DMA
            # queues start issuing transfers immediately.
            scratch = None
            if _N_WARMUP:
                scratch = const.tile([MT, 256], bf16, tag="warm_src")
                nc.vector.memset(scratch[:], 0.0)

            l8, lb, r8, rb = {}, {}, {}, {}

            def load_lhs(g, eng):
                t8 = const.tile([MT, 2, CH], f8, tag=f"l8_{g}")
                eng.dma_start(t8[:], lhs8_d[g])
                l8[g] = t8
                tb = const.tile([KB, CH], bf16, tag=f"lb_{g}")
                eng.dma_start(tb[:], lhsb_d[g])
                lb[g] = tb

            def load_rhs(j, eng):
                t8 = const.tile([MT, 2, 2, CH], f8, tag=f"r8_{j}")
                eng.dma_start(t8[:], rhs8_d[j])
                r8[j] = t8
                tb = const.tile([KB, 2, CH], bf16, tag=f"rb_{j}")
                eng.dma_start(tb[:], rhsb_d[j])
                rb[j] = tb

            # Block 0's operands race down both DMA engines in parallel —
            # chunk pair 0 is split into per-chunk pieces so ch0 lands as
            # early as possible (any idle gap between the HAM warmup and
            # the first real matmuls re-throttles the PE clock, and it has
            # been observed to stay stuck at 1.2 GHz for the whole kernel).
            r8_0 = const.tile([MT, 2, 2, CH], f8, tag="r8_0")
            rb_0 = const.tile([KB, 2, CH], bf16, tag="rb_0")
            nc.gpsimd.dma_start(r8_0[:, 0], rhs8_d[0, :, 0])
            nc.gpsimd.dma_start(rb_0[:, 0], rhsb_d[0, :, 0])
            r8[0], rb[0] = r8_0, rb_0
            load_lhs(0, nc.sync)
            bias_t = const.tile([MT, 1], f32, tag="bias")
            nc.gpsimd.dma_start(bias_t[:], bias_d[:, :])
            nc.sync.dma_start(r8_0[:, 1], rhs8_d[0, :, 1])
            nc.sync.dma_start(rb_0[:, 1], rhsb_d[0, :, 1])
            load_rhs(1, nc.gpsimd)
            load_rhs(2, nc.sync)
            load_rhs(3, nc.gpsimd)
            load_lhs(1, nc.sync)
            load_rhs(4, nc.gpsimd)
            load_rhs(5, nc.sync)
            load_rhs(6, nc.gpsimd)
            load_rhs(7, nc.sync)

            acc_t = const.tile([MT, NCOL], f32, tag="acc")
            # exp lands in SBUF (fp16) so the psum tile frees at ACT end —
            # keeping the PE dense — and the vector engine sums it off the
            # critical path with a two-port tensor_tensor_reduce (2048
            # elements in ~1024 cycles).
            exp_t = const.tile([MT, 2, NMT * CH], mybir.dt.float16, tag="exp")
            red_t = const.tile([MT, NMT * CH // 2], mybir.dt.float16, tag="red")

            # HAM warmup: dense dummy matmuls while input DMAs stream, so
            # real matmuls start at the warm PE clock.
            if _N_WARMUP:
                warm_ps = psum.tile([MT, NMT * CH], f32, tag="ps")
                for _ in range(_N_WARMUP):
                    nc.tensor.matmul(
                        warm_ps[:, :MT],
                        scratch[:, :MT],
                        scratch[:, MT:],
                        start=True,
                        stop=True,
                    )

            for g, nd in ((0, 9), (1, 8)):
                for d in range(nd):
                    ch = d if g == 0 else 8 + d
                    col = d if g == 0 else 9 + d
                    j, u = divmod(ch, 2)
                    ps = psum.tile([MT, NMT * CH], f32, tag="ps")
                    for m in range(NMT):
                        nc.tensor.matmul(
                            ps[:, m * CH:(m + 1) * CH],
                            l8[g][:, :, m * MT:(m + 1) * MT],
                            r8[j][:, u],
                            start=True,
                            stop=False,
                            perf_mode=DR,
                        )
                        nc.tensor.matmul(
                            ps[:, m * CH:(m + 1) * CH],
                            lb[g][:, m * MT:(m + 1) * MT],
                            rb[j][:, u],
                            start=False,
                            stop=True,
                        )
                    slot = col % 2
                    nc.scalar.activation(
                        exp_t[:, slot],
                        ps[:],
                        mybir.ActivationFunctionType.Exp,
                        bias=bias_t[:, 0:1],
                        scale=1.0 / (LAM * LAM),
                    )
                    half = NMT * CH // 2
                    nc.vector.scalar_tensor_tensor(
                        red_t[:],
                        exp_t[:, slot, :half],
                        1.0,
                        exp_t[:, slot, half:],
                        op0=mybir.AluOpType.mult,
                        op1=mybir.AluOpType.add,
                        accum_out=acc_t[:, col:col + 1],
                    )
            nc.sync.dma_start(acc_d[:], acc_t[:])
    nc.compile()
    _CACHE["nc"] = nc
    return nc


def _pack_inputs(s0, s1, t0, t1):
    X0 = np.concatenate([s0, t0], axis=0).astype(np.float64)
    X1 = np.concatenate([s1, t1], axis=0).astype(np.float64)

    def gamma_of(X):
        sq = np.sum(X * X, axis=1)
        sdist = 2.0 * X.shape[0] * np.sum(sq) - 2.0 * np.sum(np.sum(X, axis=0) ** 2)
        return (X.shape[0] ** 2 - X.shape[0]) / sdist, sq

    g0, sq0 = gamma_of(X0)
    g1, sq1 = gamma_of(X1)
    c = g0 * sq0 + g1 * sq1
    cbar = c.mean()
    delta16 = -16.0 * (c - cbar)
    W0 = np.sqrt(g0) * X0  # [N, 256]
    W1 = np.sqrt(g1) * X1  # [N, 64]

    L0 = np.asarray(2.0 * LAM * W0, dtype=F8)  # [N, 256] lhs fp8
    R0 = np.asarray(LAM * W0, dtype=F8)        # [N, 256] rhs fp8
    # global rhs fp8 staged [ch, p, t, n]: feature f = t*128+p
    G8 = np.ascontiguousarray(R0.reshape(NCHUNK, CH, 2, MT).transpose(0, 3, 2, 1))
    # global rhs bf16 [ch, k, n]
    Gb = np.empty((NCHUNK, KB, CH), dtype=np.float64)
    for ch in range(NCHUNK):
        rows = slice(ch * CH, (ch + 1) * CH)
        Gb[ch, :D1] = LAM * W1[rows].T
        Gb[ch, D1] = delta16[rows]
        Gb[ch, D1 + 1] = 1.0
    Gb = Gb.astype(BF)

    def lhs_for(chunk):
        rows = slice(chunk * CH, (chunk + 1) * CH)
        a8 = np.ascontiguousarray(L0[rows].reshape(CH, 2, MT).transpose(2, 1, 0))
        ab = np.empty((KB, CH), dtype=np.float64)
        ab[:D1] = 2.0 * LAM * W1[rows].T
        ab[D1] = 1.0
        ab[D1 + 1] = delta16[rows]
        return a8, ab.astype(BF)

    bias = np.full((MT, 1), -2.0 * cbar, dtype=np.float32)

    in_maps = []
    for k in range(NCORE):
        a80, ab0 = lhs_for(k)
        a81, ab1 = lhs_for((k + 8) % NCHUNK)
        order = [(k + d) % NCHUNK for d in range(NCHUNK)]
        r8 = G8[order].reshape(8, 2, MT, 2, CH).transpose(0, 2, 1, 3, 4)
        rb = Gb[order].reshape(8, 2, KB, CH).transpose(0, 2, 1, 3)
        in_maps.append({
            "lhs8": np.ascontiguousarray(np.stack([a80, a81])),
            "lhsb": np.ascontiguousarray(np.stack([ab0, ab1])),
            "rhs8": np.ascontiguousarray(r8),
            "rhsb": np.ascontiguousarray(rb),
            "bias": bias,
        })
    return in_maps


def _combine(results):
    sgn = lambda ch: 1.0 if ch < NCHUNK // 2 else -1.0
    total = 0.0
    for k in range(NCORE):
        acc = np.asarray(results[k]["acc"], dtype=np.float64)  # [128, 17]
        colsum = acc.sum(axis=0)
        for col in range(NCOL):
            if col < 9:
                d, row_chunk = col, k
            else:
                d, row_chunk = col - 9, (k + 8) % NCHUNK
            col_chunk = (row_chunk + d) % NCHUNK
            w = 1.0 if d == 0 else 2.0
            s = sgn(row_chunk) * sgn(col_chunk)
            total += w * s * colsum[col]
    return total / (B * B)


def kernel(s0, s1, t0, t1):
    global LAST_EXEC_NS, LAST_RESULTS
    nc = _build()
    in_maps = _pack_inputs(
        np.asarray(s0), np.asarray(s1), np.asarray(t0), np.asarray(t1)
    )
    trace = os.environ.get("JMMD_TRACE", "0") == "1"
    res = run_bass_kernel_spmd(nc, in_maps, core_ids=list(range(NCORE)), trace=trace)
    LAST_EXEC_NS = res.exec_time_ns
    LAST_RESULTS = res
    return np.float32(_combine(res.results))


# revision 15
# speedup vs baseline: 1.6505x; 1.0081x over previous
"""Joint Maximum Mean Discrepancy loss on 8 Trainium2 NeuronCores.

Math: for streams (s0,t0) and (s1,t1), the reference builds per-stream
Gaussian kernels K_r = exp(-gamma_r * dist_r) over feats_r = [src; tgt]
(N=8192 rows), takes their elementwise product, and returns
mean(s2s + t2t - 2*s2t) over the B x B blocks.

Device decomposition (v2 — fp8 DoubleRow + DVE reduce):
  exponent E_ij = 2*W_i.W_j - c_i - c_j with W = [sqrt(g0)*X0,
  sqrt(g1)*X1] (N x 320), c_i = |W_i|^2. Split c = cbar + delta and
  scale everything by 16 so the fp8 operands sit in e4m3's normal
  range:
    P_ij = (8W0_i).(4W0_j) + (8W1_i).(4W1_j) - 16*delta_i - 16*delta_j
    K_ij = exp(P_ij/16 - 2*cbar)
  The 256 stream-0 rows go through ONE fp8e4 DoubleRow matmul per
  m-tile (K=256 virtual in a single 512-col pass, ~2x bf16); the
  stream-1 rows + two delta rows ride a 66-row bf16 matmul. -2*cbar is
  the activation's per-partition bias AP and the 1/16 its scale, so the
  scalar engine applies them for free inside exp. Quantizing W in fp8
  perturbs source and target features identically, so the MMD
  difference cancels the quantization error (measured end-to-end rel
  err ~2e-6, better than all-bf16).

  Symmetry halves the work via a block-cyclic cover: core k owns
  row-chunks {k, k+8} (chunk = 512 rows) and computes 17 [512 x 512]
  blocks — column offsets d=0..8 from row chunk k, d=0..7 from chunk
  k+8 — every unordered off-diagonal chunk pair once (weight 2),
  diagonals once (weight 1). Per block: 8 matmuls (4 m-tiles x
  fp8-DR + bf16) into a 4-bank PSUM tile, one Exp activation over
  [128, 2048] (no accum_out — the idle vector engine does the
  [128,2048]->[128,1] block sum into acc instead, keeping the scalar
  engine's per-block cost at the pure-exp floor). Host applies
  weights/signs and the final reduction in float64.

Per-core device program (SPMD — identical instructions, data differs):
  - lhs8 [2, 128, 2, 512] fp8e4 : DoubleRow stationary, [g][p,t,r] =
    8*W0[gbase+r, t*128+p]
  - lhsb [2, 66, 512] bf16     : [8*W1^T ; ones ; -16*delta]
  - rhs8 [8, 128, 2, 2, 512] fp8e4 : chunk-PAIR tiles (one DMA each),
    [j][p,u,t,n] = 4*W0[(2j+u)*512+n, t*128+p], chunk order rotated by
    k so the program's chunk index is core-local
  - rhsb [8, 66, 2, 512] bf16  : [4*W1^T ; -16*delta ; ones]
  - bias [128, 1] f32 = -2*cbar
  - out "acc" [128, 17] f32
"""

import os

import numpy as np
import ml_dtypes

import concourse.bacc as bacc
import concourse.bass as bass
import concourse.mybir as mybir
import concourse.tile as tile
from concourse.bass_utils import run_bass_kernel_spmd

B = 4096
D0, D1 = 256, 64
N = 2 * B
CH = 512          # rows per chunk
NCHUNK = 16
NCORE = 8
MT = 128          # m-tile rows / partition count
NMT = CH // MT    # m-tiles per row-chunk (4)
NBLK = 17         # blocks per core (9 from chunk k, 8 from chunk k+8)
NCOL = NBLK       # acc columns
KB = D1 + 2       # bf16 contraction rows: 64 stream-1 + delta_j + delta_i
LAM = 4.0         # fp8 range scale on each operand; exp rescales by 1/16

F8 = ml_dtypes.float8_e4m3
BF = ml_dtypes.bfloat16

_N_WARMUP = int(os.environ.get("JMMD_WARMUP", "36"))

LAST_EXEC_NS = None
LAST_RESULTS = None

_CACHE: dict = {}


def _build():
    if "nc" in _CACHE:
        return _CACHE["nc"]
    nc = bacc.Bacc(
        "TRN2", target_bir_lowering=False, debug=False, enable_asserts=False
    )
    f32 = mybir.dt.float32
    bf16 = mybir.dt.bfloat16
    f8 = mybir.dt.float8e4
    DR = mybir.MatmulPerfMode.DoubleRow

    lhs8_d = nc.dram_tensor("lhs8", [2, MT, 2, CH], f8, kind="ExternalInput").ap()
    lhsb_d = nc.dram_tensor("lhsb", [2, KB, CH], bf16, kind="ExternalInput").ap()
    rhs8_d = nc.dram_tensor("rhs8", [8, MT, 2, 2, CH], f8, kind="ExternalInput").ap()
    rhsb_d = nc.dram_tensor("rhsb", [8, KB, 2, CH], bf16, kind="ExternalInput").ap()
    bias_d = nc.dram_tensor("bias", [MT, 1], f32, kind="ExternalInput").ap()
    acc_d = nc.dram_tensor("acc", [MT, NCOL], f32, kind="ExternalOutput").ap()

    with tile.TileContext(nc) as tc:
        with (
            tc.tile_pool(name="const", bufs=1) as const,
            tc.tile_pool(name="psum", bufs=2, space=bass.MemorySpace.PSUM) as psum,
        ):
            # warmup scratch memset on the idle vector engine so both DMA
            # queues start issuing transfers immediately.
            scratch = None
            if _N_WARMUP:
                scratch = const.tile([MT, 256], bf16, tag="warm_src")
                nc.vector.memset(scratch[:], 0.0)

            l8, lb, r8, rb = {}, {}, {}, {}

            def load_lhs(g, eng):
                t8 = const.tile([MT, 2, CH], f8, tag=f"l8_{g}")
                eng.dma_start(t8[:], lhs8_d[g])
                l8[g] = t8
                tb = const.tile([KB, CH], bf16, tag=f"lb_{g}")
                eng.dma_start(tb[:], lhsb_d[g])
                lb[g] = tb

            def load_rhs(j, eng):
                t8 = const.tile([MT, 2, 2, CH], f8, tag=f"r8_{j}")
                eng.dma_start(t8[:], rhs8_d[j])
                r8[j] = t8
                tb = const.tile([KB, 2, CH], bf16, tag=f"rb_{j}")
                eng.dma_start(tb[:], rhsb_d[j])
                rb[j] = tb

            # Block 0's operands race down both DMA engines in parallel —
            # chunk pair 0 is split into per-chunk pieces so ch0 lands as
            # early as possible (any idle gap between the HAM warmup and
            # the first real matmuls re-throttles the PE clock, and it has
            # been observed to stay stuck at 1.2 GHz for the whole kernel).
            r8_0 = const.tile([MT, 2, 2, CH], f8, tag="r8_0")
            rb_0 = const.tile([KB, 2, CH], bf16, tag="rb_0")
            nc.gpsimd.dma_start(r8_0[:, 0], rhs8_d[0, :, 0])
            nc.gpsimd.dma_start(rb_0[:, 0], rhsb_d[0, :, 0])
            r8[0], rb[0] = r8_0, rb_0
            bias_t = const.tile([MT, 1], f32, tag="bias")
            nc.sync.dma_start(bias_t[:], bias_d[:, :])
            load_lhs(0, nc.sync)
            nc.sync.dma_start(r8_0[:, 1], rhs8_d[0, :, 1])
            nc.sync.dma_start(rb_0[:, 1], rhsb_d[0, :, 1])
            load_rhs(1, nc.gpsimd)
            load_rhs(2, nc.sync)
            load_rhs(3, nc.gpsimd)
            load_lhs(1, nc.sync)
            load_rhs(4, nc.sync)
            load_rhs(5, nc.gpsimd)
            load_rhs(6, nc.sync)
            load_rhs(7, nc.gpsimd)

            acc_t = const.tile([MT, NCOL], f32, tag="acc")
            # exp lands in SBUF (fp16) so the psum tile frees at ACT end —
            # keeping the PE dense — and the vector engine sums it off the
            # critical path with a two-port tensor_tensor_reduce (2048
            # elements in ~1024 cycles).
            exp_t = const.tile([MT, 2, NMT * CH], mybir.dt.float16, tag="exp")
            red_t = const.tile([MT, NMT * CH // 2], mybir.dt.float16, tag="red")

            # HAM warmup: dense dummy matmuls while input DMAs stream, so
            # real matmuls start at the warm PE clock.
            if _N_WARMUP:
                warm_ps = psum.tile([MT, NMT * CH], f32, tag="ps")
                for _ in range(_N_WARMUP):
                    nc.tensor.matmul(
                        warm_ps[:, :MT],
                        scratch[:, :MT],
                        scratch[:, MT:],
                        start=True,
                        stop=True,
                    )

            for g, nd in ((0, 9), (1, 8)):
                for d in range(nd):
                    ch = d if g == 0 else 8 + d
                    col = d if g == 0 else 9 + d
                    j, u = divmod(ch, 2)
                    ps = psum.tile([MT, NMT * CH], f32, tag="ps")
                    for m in range(NMT):
                        nc.tensor.matmul(
                            ps[:, m * CH:(m + 1) * CH],
                            l8[g][:, :, m * MT:(m + 1) * MT],
                            r8[j][:, u],
                            start=True,
                            stop=False,
                            perf_mode=DR,
                        )
                        nc.tensor.matmul(
                            ps[:, m * CH:(m + 1) * CH],
                            lb[g][:, m * MT:(m + 1) * MT],
                            rb[j][:, u],
                            start=False,
                            stop=True,
                        )
                    slot = col % 2
                    nc.scalar.activation(
                        exp_t[:, slot],
                        ps[:],
                        mybir.ActivationFunctionType.Exp,
                        bias=bias_t[:, 0:1],
                        scale=1.0 / (LAM * LAM),
                    )
                    half = NMT * CH // 2
                    nc.vector.scalar_tensor_tensor(
                        red_t[:],
                        exp_t[:, slot, :half],
                        1.0,
                        exp_t[:, slot, half:],
                        op0=mybir.AluOpType.mult,
                        op1=mybir.AluOpType.add,
                        accum_out=acc_t[:, col:col + 1],
                    )
            nc.scalar.dma_start(acc_d[:], acc_t[:])
    nc.compile()
    _CACHE["nc"] = nc
    return nc


def _pack_inputs(s0, s1, t0, t1):
    X0 = np.concatenate([s0, t0], axis=0).astype(np.float64)
    X1 = np.concatenate([s1, t1], axis=0).astype(np.float64)

    def gamma_of(X):
        sq = np.sum(X * X, axis=1)
        sdist = 2.0 * X.shape[0] * np.sum(sq) - 2.0 * np.sum(np.sum(X, axis=0) ** 2)
        return (X.shape[0] ** 2 - X.shape[0]) / sdist, sq

    g0, sq0 = gamma_of(X0)
    g1, sq1 = gamma_of(X1)
    c = g0 * sq0 + g1 * sq1
    cbar = c.mean()
    delta16 = -16.0 * (c - cbar)
    W0 = np.sqrt(g0) * X0  # [N, 256]
    W1 = np.sqrt(g1) * X1  # [N, 64]

    L0 = np.asarray(2.0 * LAM * W0, dtype=F8)  # [N, 256] lhs fp8
    R0 = np.asarray(LAM * W0, dtype=F8)        # [N, 256] rhs fp8
    # global rhs fp8 staged [ch, p, t, n]: feature f = t*128+p
    G8 = np.ascontiguousarray(R0.reshape(NCHUNK, CH, 2, MT).transpose(0, 3, 2, 1))
    # global rhs bf16 [ch, k, n]
    Gb = np.empty((NCHUNK, KB, CH), dtype=np.float64)
    for ch in range(NCHUNK):
        rows = slice(ch * CH, (ch + 1) * CH)
        Gb[ch, :D1] = LAM * W1[rows].T
        Gb[ch, D1] = delta16[rows]
        Gb[ch, D1 + 1] = 1.0
    Gb = Gb.astype(BF)

    def lhs_for(chunk):
        rows = slice(chunk * CH, (chunk + 1) * CH)
        a8 = np.ascontiguousarray(L0[rows].reshape(CH, 2, MT).transpose(2, 1, 0))
        ab = np.empty((KB, CH), dtype=np.float64)
        ab[:D1] = 2.0 * LAM * W1[rows].T
        ab[D1] = 1.0
        ab[D1 + 1] = delta16[rows]
        return a8, ab.astype(BF)

    bias = np.full((MT, 1), -2.0 * cbar, dtype=np.float32)

    in_maps = []
    for k in range(NCORE):
        a80, ab0 = lhs_for(k)
        a81, ab1 = lhs_for((k + 8) % NCHUNK)
        order = [(k + d) % NCHUNK for d in range(NCHUNK)]
        r8 = G8[order].reshape(8, 2, MT, 2, CH).transpose(0, 2, 1, 3, 4)
        rb = Gb[order].reshape(8, 2, KB, CH).transpose(0, 2, 1, 3)
        in_maps.append({
            "lhs8": np.ascontiguousarray(np.stack([a80, a81])),
            "lhsb": np.ascontiguousarray(np.stack([ab0, ab1])),
            "rhs8": np.ascontiguousarray(r8),
            "rhsb": np.ascontiguousarray(rb),
            "bias": bias,
        })
    return in_maps


def _combine(results):
    sgn = lambda ch: 1.0 if ch < NCHUNK // 2 else -1.0
    total = 0.0
    for k in range(NCORE):
        acc = np.asarray(results[k]["acc"], dtype=np.float64)  # [128, 17]
        colsum = acc.sum(axis=0)
        for col in range(NCOL):
            if col < 9:
                d, row_chunk = col, k
            else:
                d, row_chunk = col - 9, (k + 8) % NCHUNK
            col_chunk = (row_chunk + d) % NCHUNK
            w = 1.0 if d == 0 else 2.0
            s = sgn(row_chunk) * sgn(col_chunk)
            total += w * s * colsum[col]
    return total / (B * B)


def kernel(s0, s1, t0, t1):
    global LAST_EXEC_NS, LAST_RESULTS
    nc = _build()
    in_maps = _pack_inputs(
        np.asarray(s0), np.asarray(s1), np.asarray(t0), np.asarray(t1)
    )
    trace = os.environ.get("JMMD_TRACE", "0") == "1"
    res = run_bass_kernel_spmd(nc, in_maps, core_ids=list(range(NCORE)), trace=trace)
    LAST_EXEC_NS = res.exec_time_ns
    LAST_RESULTS = res
    return np.float32(_combine(res.results))
